# revision 21
# baseline (speedup 1.0000x reference)
"""Trainium2 Bass kernel for nn_AttentionConv (B=4,H=W=64,C=128,heads=2).

Sharding: 8 cores = (batch b in 0..3) x (query-half qh in 0..1).
Each core computes full attention for its 2048 query pixels of batch b,
over all 4096 keys, both heads, plus the qkv and output 1x1-conv
projections.  No cross-core communication.

Host-side layout prep (part of sharding):
 - x[b] is transposed to [C, N] and rotated so this core's queries are
   columns 0..2047 (key order is softmax-invariant).
 - w_qkv is split into wq/wk/wv, each [128, 128] = both heads' 64 dims
   column-concatenated.

Per-core device algorithm (matmuls bf16, accumulate f32):
 - QT = wq^T x^T  -> [128(2h x 64d), 2048];  KT = wk^T x^T -> [128, 4096]
 - V natural [4096, 128] per key-chunk as lhsT tiles [128, 2, 128]:
   cols 0..63 = v_head, cols 64..127 = 1.0 (ones columns make the PV
   matmul also emit the softmax row-sum Z on partitions 64..127).
 - per query-group (512) x key-chunk (128): S^T = K Q^T in PSUM (both
   heads packed: disjoint PE row-groups + PSUM banks, run concurrently).
 - exp is SPLIT across two engines: ~2/3 of key chunks on ScalarE
   (ACT exp, scale=C^-0.5 folded in) and ~1/3 on VectorE via a
   Schraudolph-style bit trick: one tensor_scalar computes
   round(s*A + B) into int16 whose bits ARE the bf16 exp(s*SCALE)
   (A = SCALE*128/ln2, B = 128*(127-c)).  DVE f32->i16 converts RNE
   (hardware-probed); rel-err impact measured at <2e-3 end to end.
 - PV accumulates O' and Z in one PSUM tile per group (ones trick);
   epilogue uses reciprocal_approx_fast (1 DVE op, ~5x faster than the
   iterative divide) then rt = O' * rz; out = rt^T w_out + b_out.

PSUM budget (16KB/partition, exact): st pool 3 x [128,1024] f32 (12KB,
shared rotation also carrying projection/outproj/bias psum tiles) +
o_ps 1 x [128,1024] f32 (4KB, heads packed on free dim, single buffer
-- the epilogue drains it before the next group's first PV, which is
emitted 10 chunks late to guarantee the WAR gap without stalling PE).
"""

import math
import numpy as np

import concourse.bass as bass
import concourse.tile as tile
from concourse.tile import add_dep_helper
from concourse import bacc, mybir
from concourse.bass_utils import run_bass_kernel_spmd

F32 = mybir.dt.float32
BF16 = mybir.dt.bfloat16
I16 = mybir.dt.int16

B = 4
C = 128
NPIX = 4096          # 64*64 pixels per batch
NQ = 2048            # queries per core (half batch)
HC = 64              # head dim
KC = 128             # key chunk
NKC = NPIX // KC     # 32
QG = 512             # query group (per head; ST tile packs both heads)
NQG = NQ // QG       # 4
SCALE = float(C) ** -0.5
N_CORES = 8

# Schraudolph bf16-bit exp constants (DVE path)
C_MAGIC = 0.045
A_MAGIC = SCALE * 128.0 / math.log(2.0)
B_MAGIC = 128.0 * (127.0 - C_MAGIC)
# key chunks computed on DVE instead of ACT (per group)
DVE_KC = frozenset({8, 10, 18, 20, 22, 24, 26, 28, 30, 31})
DVE_KC_LAST = DVE_KC

LAG = 6              # PV emission lags ST/exp by this many chunks
FIRST_PV = 10        # earliest within-group step for a group's first PV

_CACHE = {}


def _build_nc():
    nc = bacc.Bacc("TRN2", target_bir_lowering=False, debug=False)

    xt_d = nc.dram_tensor("xt", [C, NPIX], F32, kind="ExternalInput")
    wq_d = nc.dram_tensor("wq", [C, 128], F32, kind="ExternalInput")
    wk_d = nc.dram_tensor("wk", [C, 128], F32, kind="ExternalInput")
    wv_d = nc.dram_tensor("wv", [C, 128], F32, kind="ExternalInput")
    wo_d = nc.dram_tensor("wo", [C, C], F32, kind="ExternalInput")
    bo_d = nc.dram_tensor("bo", [1, C], F32, kind="ExternalInput")
    out_d = nc.dram_tensor("out", [NQ, C], F32, kind="ExternalOutput")

    Exp = mybir.ActivationFunctionType.Exp

    with tile.TileContext(nc) as tc:
        with (
            tc.tile_pool(name="const", bufs=1) as const,
            tc.tile_pool(name="stage", bufs=4) as stage,
            tc.tile_pool(name="et", bufs=12) as etp,
            tc.tile_pool(name="rz", bufs=2) as rzp,
            tc.tile_pool(name="osb", bufs=2) as osbp,
            tc.tile_pool(name="st", bufs=1, space="PSUM") as stp,
            tc.tile_pool(name="op", bufs=1, space="PSUM") as opp,
        ):
            # ---- persistent SBUF tensors
            xtb = [const.tile([C, 512], BF16, tag=f"xtb{j}", name=f"xtb{j}")
                   for j in range(8)]
            qt = [const.tile([128, 512], BF16, tag=f"qt{j}", name=f"qt{j}")
                  for j in range(4)]
            kt = [const.tile([128, 512], BF16, tag=f"kt{j}", name=f"kt{j}")
                  for j in range(8)]
            v4 = [const.tile([128, 2, 128], BF16, tag=f"v4_{k}",
                             name=f"v4_{k}") for k in range(NKC)]
            rt = const.tile([128, NQ], BF16, tag="rt")
            bias_bc = const.tile([128, C], F32, tag="bias_bc")
            wqb = const.tile([C, 128], BF16, tag="wqb")
            wkb = const.tile([C, 128], BF16, tag="wkb")
            wvb = const.tile([C, 128], BF16, tag="wvb")
            wob = const.tile([C, C], BF16, tag="wob")
            ones1 = const.tile([1, C], F32, tag="ones1")
            onesb = const.tile([1, C], BF16, tag="onesb")
            warm = const.tile([1, 2], F32, tag="warm")

            # dummy exp first: loads the ACT table set off the critical path
            nc.vector.memset(warm[:], 0.0)
            nc.scalar.activation(warm[:], warm[:], Exp)

            # PE warm-up while DMAs run (HAM clock-gate to K=8/8)
            junk = const.tile([C, 512], BF16, tag="junk")
            nc.vector.memset(junk[:], 0.25)
            wst = stp.tile([128, 2 * QG], F32, tag="st", bufs=3, name="warm_st")
            for w in range(8):
                nc.tensor.matmul(wst[:, 0:512], junk[:, 0:128], junk[:],
                                 start=True, stop=True)

            # DMAs: first x chunk, then weights needed early, then the rest
            xs = [stage.tile([C, 512], F32, tag=f"xs{j}", name=f"xs{j}")
                  for j in range(8)]
            nc.sync.dma_start(xs[0][:], xt_d[:, 0:512])
            for name, dram, dst in (("wq", wq_d, wqb), ("wk", wk_d, wkb),
                                    ("wv", wv_d, wvb)):
                w32 = stage.tile([C, 128], F32, tag="w32", name=f"w32_{name}")
                nc.sync.dma_start(w32[:], dram[:])
                nc.vector.tensor_copy(dst[:], w32[:])
            bo32 = stage.tile([1, C], F32, tag="bo32")
            nc.sync.dma_start(bo32[:], bo_d[:])
            wo32 = stage.tile([C, C], F32, tag="wo32", name="wo32")
            nc.sync.dma_start(wo32[:], wo_d[:])
            for j in range(1, 8):
                nc.sync.dma_start(xs[j][:], xt_d[:, j * 512:(j + 1) * 512])

            def emit_proj_piece(j, piece):
                # projections for one 512-pixel chunk, split into 4 pieces
                # emitted on consecutive steps so PE/DVE stay smooth
                if piece == 0:
                    nc.vector.tensor_copy(xtb[j][:], xs[j][:])
                    if j < 4:  # QT over local queries
                        p = stp.tile([128, 512], F32, tag="st", bufs=3,
                                     name=f"pq{j}")
                        nc.tensor.matmul(p[:], wqb[:], xtb[j][:],
                                         start=True, stop=True)
                        nc.vector.tensor_copy(qt[j][:], p[:])
                    return
                if piece == 1:
                    p = stp.tile([128, 512], F32, tag="st", bufs=3,
                                 name=f"pk{j}")
                    nc.tensor.matmul(p[:], wkb[:], xtb[j][:],
                                     start=True, stop=True)
                    nc.vector.tensor_copy(kt[j][:], p[:])
                    return
                for kq in (0, 1) if piece == 2 else (2, 3):
                    k = j * 4 + kq
                    pv = stp.tile([128, 128], F32, tag="st", bufs=3,
                                  name=f"pvj{k}")
                    nc.tensor.matmul(pv[:],
                                     xtb[j][:, kq * 128:(kq + 1) * 128],
                                     wvb[:], start=True, stop=True)
                    nc.vector.memset(v4[k][:, :, 64:128], 1.0)
                    nc.vector.tensor_copy(
                        v4[k][:, :, 0:64],
                        pv[:].rearrange("p (h d) -> p h d", h=2))

            def emit_proj_chunk(j):
                for piece in range(4):
                    emit_proj_piece(j, piece)

            emit_proj_chunk(0)
            nc.vector.memset(ones1[:], 1.0)
            nc.vector.memset(onesb[:], 1.0)

            # ---- attention ----
            o_ps = {}     # group -> PSUM accumulation tile
            ets = {}      # global chunk -> et tile

            zrow = {}

            def emit_drain(g):
                # the Z row first (it gates the Z-transpose matmuls)
                zr = rzp.tile([1, 2 * QG], BF16, tag="zrow", name=f"zrow_{g}")
                nc.vector.tensor_copy(zr[:], o_ps[g][64:65, :])
                zrow[g] = zr
                emit_drain_rt(g, 0)
                emit_drain_rt(g, 1)

            def emit_drain_rt(g, h):
                q0 = g * QG
                nc.vector.tensor_copy(
                    rt[h * HC:(h + 1) * HC, q0:q0 + QG],
                    o_ps[g][0:64, h * QG:(h + 1) * QG])

            fin = {}

            def emit_finish_a(g):
                # Z^T via K=1 matmuls (the matmul transposes a row into a
                # column), one [128,8] recip, then the outproj matmuls
                q0 = g * QG
                zt = stp.tile([128, 8], F32, tag="st", bufs=3,
                              name=f"zt_{g}")
                for j in range(8):
                    nc.tensor.matmul(zt[:, j:j + 1],
                                     zrow[g][0:1, j * 128:(j + 1) * 128],
                                     onesb[0:1, 0:1], start=True, stop=True)
                rz8 = rzp.tile([128, 8], F32, tag="rz8", name=f"rz8_{g}")
                nc.vector.reciprocal(rz8[:], zt[:])
                gp = stp.tile([128, 2 * QG], F32, tag="st", bufs=3,
                              name=f"gps_{g}")
                for i in range(4):
                    for h in range(2):
                        nc.tensor.matmul(
                            gp[:, h * QG + i * 128:h * QG + (i + 1) * 128],
                            rt[h * HC:(h + 1) * HC,
                               q0 + i * 128:q0 + (i + 1) * 128],
                            wob[h * HC:(h + 1) * HC, :],
                            start=True, stop=True)
                ob = osbp.tile([128, 512], F32, tag="osb", name=f"ob_{g}")
                last = g == NQG - 1
                for i in range(4):   # independent first halves (h0 + bias)
                    nc.vector.scalar_tensor_tensor(
                        ob[:, i * 128:(i + 1) * 128],
                        gp[:, i * 128:(i + 1) * 128],
                        rz8[:, i:i + 1], bias_bc[:],
                        mybir.AluOpType.mult, mybir.AluOpType.add)
                for i in range(4):   # second halves, chain distance 4 ops
                    obi = ob[:, i * 128:(i + 1) * 128]
                    nc.vector.scalar_tensor_tensor(
                        obi, gp[:, QG + i * 128:QG + (i + 1) * 128],
                        rz8[:, 4 + i:5 + i], obi,
                        mybir.AluOpType.mult, mybir.AluOpType.add)
                    if last:    # per-block DMA so the tail drains early,
                        # alternating issue queues to overlap descriptor gen
                        eng = nc.sync if i % 2 == 0 else nc.gpsimd
                        eng.dma_start(
                            out_d[q0 + i * 128:q0 + (i + 1) * 128, :], obi)
                if not last:
                    nc.sync.dma_start(
                        out_d[q0:q0 + QG, :].rearrange(
                            "(c r) w -> r c w", r=128),
                        ob[:].rearrange("p (c w) -> p c w", w=128))

            def emit_st_exp(g, kc, t):
                st = stp.tile([128, 2 * QG], F32, tag="st", bufs=3,
                              name=f"st_{g}_{kc}")
                ktt = kt[kc // 4]
                ks = slice((kc % 4) * 128, (kc % 4 + 1) * 128)
                for h in range(2):
                    hp = slice(h * HC, (h + 1) * HC)
                    nc.tensor.matmul(
                        st[:, h * QG:(h + 1) * QG],
                        ktt[hp, ks], qt[g][hp, :],
                        start=True, stop=True)
                et = etp.tile([128, 2 * QG], BF16, tag="et",
                              name=f"et_{g}_{kc}")
                if kc in (DVE_KC_LAST if g == NQG - 1 else DVE_KC):
                    nc.vector.tensor_scalar(
                        et[:].bitcast(I16), st[:],
                        float(A_MAGIC), float(B_MAGIC),
                        mybir.AluOpType.mult, mybir.AluOpType.add)
                else:
                    nc.scalar.activation(et[:], st[:], Exp, scale=SCALE)
                ets[t] = et

            def emit_pv(p):
                g, pk = p // NKC, p % NKC
                if p % NKC == 0:
                    o_ps[g] = opp.tile([128, 2 * QG], F32, tag="ops",
                                       name=f"o_ps_{g}")
                et = ets.pop(p)
                for h in range(2):
                    nc.tensor.matmul(
                        o_ps[g][:, h * QG:(h + 1) * QG], v4[pk][:, h, :],
                        et[:, h * QG:(h + 1) * QG],
                        start=(pk == 0), stop=(pk == NKC - 1))

            # PV(p) emission step: lag LAG behind ST, but never before
            # within-group step FIRST_PV (o_ps single-buffer WAR gap)
            pv_at = {}
            for p in range(NQG * NKC):
                t = max(p + LAG, (p // NKC) * NKC + FIRST_PV)
                pv_at.setdefault(t, []).append(p)

            # STs are emitted in batches of 3 then their PVs: ST-pair ->
            # ST-pair transitions chain at the PE issue floor (~215ns) while
            # mixed PV<->ST transitions each cost ~+100ns of array turnaround
            T_END = NQG * NKC + LAG
            for t0 in range(0, T_END, 3):
                batch = range(t0, min(t0 + 3, T_END))
                for t in batch:
                    if t < NQG * NKC:
                        emit_st_exp(t // NKC, t % NKC, t)
                for t in batch:
                    g, kc = t // NKC, t % NKC
                    for p in pv_at.get(t, ()):
                        emit_pv(p)
                        if p % NKC == NKC - 1:  # group fully summed
                            emit_drain(p // NKC)
                    if t == 2:
                        nc.vector.tensor_copy(wob[:], wo32[:])
                    if t == 20:
                        bps = stp.tile([128, C], F32, tag="st", bufs=3,
                                       name="bps")
                        nc.tensor.matmul(bps[:], ones1[:], bo32[:],
                                         start=True, stop=True)
                        nc.vector.tensor_copy(bias_bc[:], bps[:])
                    if t < NQG * NKC and g == 0 and t < 28:
                        emit_proj_piece(1 + t // 4, t % 4)
                    if t < NQG * NKC and g >= 1 and kc == 12:
                        emit_finish_a(g - 1)
            emit_finish_a(NQG - 1)

    nc.compile()
    return nc


def _prep_in_maps(x, w_qkv, w_out, b_out):
    x = np.asarray(x, dtype=np.float32).reshape(B, NPIX, C)
    w_qkv = np.asarray(w_qkv, dtype=np.float32)
    w_out = np.asarray(w_out, dtype=np.float32)
    b_out = np.asarray(b_out, dtype=np.float32)

    wq = np.ascontiguousarray(
        np.concatenate([w_qkv[:, 0:64], w_qkv[:, 192:256]], axis=1))
    wk = np.ascontiguousarray(
        np.concatenate([w_qkv[:, 64:128], w_qkv[:, 256:320]], axis=1))
    wv = np.ascontiguousarray(
        np.concatenate([w_qkv[:, 128:192], w_qkv[:, 320:384]], axis=1))
    wo = np.ascontiguousarray(w_out)
    bo = np.ascontiguousarray(b_out.reshape(1, C))

    in_maps = []
    for core in range(N_CORES):
        b, qh = core // 2, core % 2
        xbT = x[b].T                     # [C, NPIX]
        q0 = qh * NQ
        xt = np.ascontiguousarray(
            np.concatenate([xbT[:, q0:], xbT[:, :q0]], axis=1))
        in_maps.append({"xt": xt, "wq": wq, "wk": wk, "wv": wv,
                        "wo": wo, "bo": bo})
    return in_maps


def run(x, w_qkv, w_out, b_out, trace=False, **run_kwargs):
    if "nc" not in _CACHE:
        _CACHE["nc"] = _build_nc()
    nc = _CACHE["nc"]
    in_maps = _prep_in_maps(x, w_qkv, w_out, b_out)
    res = run_bass_kernel_spmd(nc, in_maps, core_ids=list(range(N_CORES)),
                               trace=trace, **run_kwargs)
    out = np.empty((B, NPIX, C), dtype=np.float32)
    for core in range(N_CORES):
        b, qh = core // 2, core % 2
        out[b, qh * NQ:(qh + 1) * NQ, :] = res.results[core]["out"]
    return out.reshape(B, 64, 64, C), res


def kernel(x, w_qkv, w_out, b_out):
    out, _ = run(x, w_qkv, w_out, b_out, trace=False)
    return out


# revision 22
# speedup vs baseline: 1.0003x; 1.0003x over previous
"""Trainium2 Bass kernel for nn_AttentionConv (B=4,H=W=64,C=128,heads=2).

Sharding: 8 cores = (batch b in 0..3) x (query-half qh in 0..1).
Each core computes full attention for its 2048 query pixels of batch b,
over all 4096 keys, both heads, plus the qkv and output 1x1-conv
projections.  No cross-core communication.

Host-side layout prep (part of sharding):
 - x[b] is transposed to [C, N] and rotated so this core's queries are
   columns 0..2047 (key order is softmax-invariant).
 - w_qkv is split into wq/wk/wv, each [128, 128] = both heads' 64 dims
   column-concatenated.

Per-core device algorithm (matmuls bf16, accumulate f32):
 - QT = wq^T x^T  -> [128(2h x 64d), 2048];  KT = wk^T x^T -> [128, 4096]
 - V natural [4096, 128] per key-chunk as lhsT tiles [128, 2, 128]:
   cols 0..63 = v_head, cols 64..127 = 1.0 (ones columns make the PV
   matmul also emit the softmax row-sum Z on partitions 64..127).
 - per query-group (512) x key-chunk (128): S^T = K Q^T in PSUM (both
   heads packed: disjoint PE row-groups + PSUM banks, run concurrently).
 - exp is SPLIT across two engines: ~2/3 of key chunks on ScalarE
   (ACT exp, scale=C^-0.5 folded in) and ~1/3 on VectorE via a
   Schraudolph-style bit trick: one tensor_scalar computes
   round(s*A + B) into int16 whose bits ARE the bf16 exp(s*SCALE)
   (A = SCALE*128/ln2, B = 128*(127-c)).  DVE f32->i16 converts RNE
   (hardware-probed); rel-err impact measured at <2e-3 end to end.
 - PV accumulates O' and Z in one PSUM tile per group (ones trick);
   normalization is commuted past the output projection: Z^T via K=1
   matmuls (the matmul transposes a row to a column), one tiny [128,8]
   reciprocal, out = (O'^T w_out) * (1/Z) + b via per-partition-scalar
   scalar_tensor_tensor combines.

PSUM budget (16KB/partition, exact): st pool 3 x [128,1024] f32 (12KB,
shared rotation also carrying projection/outproj/bias psum tiles) +
o_ps 1 x [128,1024] f32 (4KB, heads packed on free dim, single buffer
-- the epilogue drains it before the next group's first PV, which is
emitted 10 chunks late to guarantee the WAR gap without stalling PE).
"""

import math
import numpy as np

import concourse.bass as bass
import concourse.tile as tile
from concourse.tile import add_dep_helper
from concourse import bacc, mybir
from concourse.bass_utils import run_bass_kernel_spmd

F32 = mybir.dt.float32
BF16 = mybir.dt.bfloat16
I16 = mybir.dt.int16

B = 4
C = 128
NPIX = 4096          # 64*64 pixels per batch
NQ = 2048            # queries per core (half batch)
HC = 64              # head dim
KC = 128             # key chunk
NKC = NPIX // KC     # 32
QG = 512             # query group (per head; ST tile packs both heads)
NQG = NQ // QG       # 4
SCALE = float(C) ** -0.5
N_CORES = 8

# Schraudolph bf16-bit exp constants (DVE path)
C_MAGIC = 0.045
A_MAGIC = SCALE * 128.0 / math.log(2.0)
B_MAGIC = 128.0 * (127.0 - C_MAGIC)
# key chunks computed on DVE instead of ACT (per group)
DVE_KC = frozenset({8, 10, 18, 20, 22, 24, 26, 28, 30, 31})
DVE_KC_LAST = DVE_KC

LAG = 6              # PV emission lags ST/exp by this many chunks
FIRST_PV = 10        # earliest within-group step for a group's first PV

_CACHE = {}


def _build_nc():
    nc = bacc.Bacc("TRN2", target_bir_lowering=False, debug=False)

    xt_d = nc.dram_tensor("xt", [C, NPIX], F32, kind="ExternalInput")
    wq_d = nc.dram_tensor("wq", [C, 128], F32, kind="ExternalInput")
    wk_d = nc.dram_tensor("wk", [C, 128], F32, kind="ExternalInput")
    wv_d = nc.dram_tensor("wv", [C, 128], F32, kind="ExternalInput")
    wo_d = nc.dram_tensor("wo", [C, C], F32, kind="ExternalInput")
    bo_d = nc.dram_tensor("bo", [1, C], F32, kind="ExternalInput")
    out_d = nc.dram_tensor("out", [NQ, C], F32, kind="ExternalOutput")

    Exp = mybir.ActivationFunctionType.Exp

    with tile.TileContext(nc) as tc:
        with (
            tc.tile_pool(name="const", bufs=1) as const,
            tc.tile_pool(name="stage", bufs=4) as stage,
            tc.tile_pool(name="et", bufs=12) as etp,
            tc.tile_pool(name="rz", bufs=2) as rzp,
            tc.tile_pool(name="osb", bufs=2) as osbp,
            tc.tile_pool(name="st", bufs=1, space="PSUM") as stp,
            tc.tile_pool(name="op", bufs=1, space="PSUM") as opp,
        ):
            # ---- persistent SBUF tensors
            xtb = [const.tile([C, 512], BF16, tag=f"xtb{j}", name=f"xtb{j}")
                   for j in range(8)]
            qt = [const.tile([128, 512], BF16, tag=f"qt{j}", name=f"qt{j}")
                  for j in range(4)]
            kt = [const.tile([128, 512], BF16, tag=f"kt{j}", name=f"kt{j}")
                  for j in range(8)]
            v4 = [const.tile([128, 2, 128], BF16, tag=f"v4_{k}",
                             name=f"v4_{k}") for k in range(NKC)]
            rt = const.tile([128, NQ], BF16, tag="rt")
            bias_bc = const.tile([128, C], F32, tag="bias_bc")
            wqb = const.tile([C, 128], BF16, tag="wqb")
            wkb = const.tile([C, 128], BF16, tag="wkb")
            wvb = const.tile([C, 128], BF16, tag="wvb")
            wob = const.tile([C, C], BF16, tag="wob")
            ones1 = const.tile([1, C], F32, tag="ones1")
            onesb = const.tile([1, C], BF16, tag="onesb")
            warm = const.tile([1, 2], F32, tag="warm")

            # dummy exp first: loads the ACT table set off the critical path
            nc.vector.memset(warm[:], 0.0)
            nc.scalar.activation(warm[:], warm[:], Exp)

            # PE warm-up while DMAs run (HAM clock-gate to K=8/8)
            junk = const.tile([C, 512], BF16, tag="junk")
            nc.vector.memset(junk[:], 0.25)
            wst = stp.tile([128, 2 * QG], F32, tag="st", bufs=3, name="warm_st")
            for w in range(8):
                nc.tensor.matmul(wst[:, 0:512], junk[:, 0:128], junk[:],
                                 start=True, stop=True)

            # DMAs: first x chunk, then weights needed early, then the rest
            xs = [stage.tile([C, 512], F32, tag=f"xs{j}", name=f"xs{j}")
                  for j in range(8)]
            nc.sync.dma_start(xs[0][:], xt_d[:, 0:512])
            for name, dram, dst in (("wq", wq_d, wqb), ("wk", wk_d, wkb),
                                    ("wv", wv_d, wvb)):
                w32 = stage.tile([C, 128], F32, tag="w32", name=f"w32_{name}")
                nc.sync.dma_start(w32[:], dram[:])
                nc.vector.tensor_copy(dst[:], w32[:])
            bo32 = stage.tile([1, C], F32, tag="bo32")
            nc.sync.dma_start(bo32[:], bo_d[:])
            wo32 = stage.tile([C, C], F32, tag="wo32", name="wo32")
            nc.sync.dma_start(wo32[:], wo_d[:])
            for j in range(1, 8):
                nc.sync.dma_start(xs[j][:], xt_d[:, j * 512:(j + 1) * 512])

            def emit_proj_piece(j, piece):
                # projections for one 512-pixel chunk, split into 4 pieces
                # emitted on consecutive steps so PE/DVE stay smooth
                if piece == 0:
                    nc.vector.tensor_copy(xtb[j][:], xs[j][:])
                    if j < 4:  # QT over local queries
                        p = stp.tile([128, 512], F32, tag="st", bufs=3,
                                     name=f"pq{j}")
                        nc.tensor.matmul(p[:], wqb[:], xtb[j][:],
                                         start=True, stop=True)
                        nc.vector.tensor_copy(qt[j][:], p[:])
                    return
                if piece == 1:
                    p = stp.tile([128, 512], F32, tag="st", bufs=3,
                                 name=f"pk{j}")
                    nc.tensor.matmul(p[:], wkb[:], xtb[j][:],
                                     start=True, stop=True)
                    nc.vector.tensor_copy(kt[j][:], p[:])
                    return
                for kq in (0, 1) if piece == 2 else (2, 3):
                    k = j * 4 + kq
                    pv = stp.tile([128, 128], F32, tag="st", bufs=3,
                                  name=f"pvj{k}")
                    nc.tensor.matmul(pv[:],
                                     xtb[j][:, kq * 128:(kq + 1) * 128],
                                     wvb[:], start=True, stop=True)
                    nc.vector.memset(v4[k][:, :, 64:128], 1.0)
                    nc.vector.tensor_copy(
                        v4[k][:, :, 0:64],
                        pv[:].rearrange("p (h d) -> p h d", h=2))

            def emit_proj_chunk(j):
                for piece in range(4):
                    emit_proj_piece(j, piece)

            emit_proj_chunk(0)
            nc.vector.memset(ones1[:], 1.0)
            nc.vector.memset(onesb[:], 1.0)

            # ---- attention ----
            o_ps = {}     # group -> PSUM accumulation tile
            ets = {}      # global chunk -> et tile

            zrow = {}

            def emit_drain(g):
                # the Z row first (it gates the Z-transpose matmuls)
                zr = rzp.tile([1, 2 * QG], BF16, tag="zrow", name=f"zrow_{g}")
                nc.vector.tensor_copy(zr[:], o_ps[g][64:65, :])
                zrow[g] = zr
                emit_drain_rt(g, 0)
                emit_drain_rt(g, 1)

            def emit_drain_rt(g, h):
                q0 = g * QG
                nc.vector.tensor_copy(
                    rt[h * HC:(h + 1) * HC, q0:q0 + QG],
                    o_ps[g][0:64, h * QG:(h + 1) * QG])

            fin = {}

            def emit_finish_a(g):
                # Z^T via K=1 matmuls (the matmul transposes a row into a
                # column), one [128,8] recip, then the outproj matmuls
                q0 = g * QG
                zt = stp.tile([128, 8], F32, tag="st", bufs=3,
                              name=f"zt_{g}")
                for j in range(8):
                    nc.tensor.matmul(zt[:, j:j + 1],
                                     zrow[g][0:1, j * 128:(j + 1) * 128],
                                     onesb[0:1, 0:1], start=True, stop=True)
                rz8 = rzp.tile([128, 8], F32, tag="rz8", name=f"rz8_{g}")
                nc.vector.reciprocal(rz8[:], zt[:])
                gp = stp.tile([128, 2 * QG], F32, tag="st", bufs=3,
                              name=f"gps_{g}")
                for i in range(4):
                    for h in range(2):
                        nc.tensor.matmul(
                            gp[:, h * QG + i * 128:h * QG + (i + 1) * 128],
                            rt[h * HC:(h + 1) * HC,
                               q0 + i * 128:q0 + (i + 1) * 128],
                            wob[h * HC:(h + 1) * HC, :],
                            start=True, stop=True)
                ob = osbp.tile([128, 512], F32, tag="osb", name=f"ob_{g}")
                last = g == NQG - 1
                for i in range(4):   # independent first halves (h0 + bias)
                    nc.vector.scalar_tensor_tensor(
                        ob[:, i * 128:(i + 1) * 128],
                        gp[:, i * 128:(i + 1) * 128],
                        rz8[:, i:i + 1], bias_bc[:],
                        mybir.AluOpType.mult, mybir.AluOpType.add)
                for i in range(4):   # second halves, chain distance 4 ops
                    obi = ob[:, i * 128:(i + 1) * 128]
                    nc.vector.scalar_tensor_tensor(
                        obi, gp[:, QG + i * 128:QG + (i + 1) * 128],
                        rz8[:, 4 + i:5 + i], obi,
                        mybir.AluOpType.mult, mybir.AluOpType.add)
                    if last:    # per-block DMA so the tail drains early,
                        # alternating issue queues to overlap descriptor gen
                        eng = nc.sync if i % 2 == 0 else nc.gpsimd
                        eng.dma_start(
                            out_d[q0 + i * 128:q0 + (i + 1) * 128, :], obi)
                if not last:
                    nc.sync.dma_start(
                        out_d[q0:q0 + QG, :].rearrange(
                            "(c r) w -> r c w", r=128),
                        ob[:].rearrange("p (c w) -> p c w", w=128))

            def emit_st_exp(g, kc, t):
                st = stp.tile([128, 2 * QG], F32, tag="st", bufs=3,
                              name=f"st_{g}_{kc}")
                ktt = kt[kc // 4]
                ks = slice((kc % 4) * 128, (kc % 4 + 1) * 128)
                for h in range(2):
                    hp = slice(h * HC, (h + 1) * HC)
                    nc.tensor.matmul(
                        st[:, h * QG:(h + 1) * QG],
                        ktt[hp, ks], qt[g][hp, :],
                        start=True, stop=True)
                et = etp.tile([128, 2 * QG], BF16, tag="et",
                              name=f"et_{g}_{kc}")
                if kc in (DVE_KC_LAST if g == NQG - 1 else DVE_KC):
                    nc.vector.tensor_scalar(
                        et[:].bitcast(I16), st[:],
                        float(A_MAGIC), float(B_MAGIC),
                        mybir.AluOpType.mult, mybir.AluOpType.add)
                else:
                    nc.scalar.activation(et[:], st[:], Exp, scale=SCALE)
                ets[t] = et

            def emit_pv(p):
                g, pk = p // NKC, p % NKC
                if p % NKC == 0:
                    o_ps[g] = opp.tile([128, 2 * QG], F32, tag="ops",
                                       name=f"o_ps_{g}")
                et = ets.pop(p)
                for h in range(2):
                    nc.tensor.matmul(
                        o_ps[g][:, h * QG:(h + 1) * QG], v4[pk][:, h, :],
                        et[:, h * QG:(h + 1) * QG],
                        start=(pk == 0), stop=(pk == NKC - 1))

            # PV(p) emission step: lag LAG behind ST, but never before
            # within-group step FIRST_PV (o_ps single-buffer WAR gap)
            pv_at = {}
            for p in range(NQG * NKC):
                t = max(p + LAG, (p // NKC) * NKC + FIRST_PV)
                pv_at.setdefault(t, []).append(p)

            # STs are emitted in batches of 3 then their PVs: ST-pair ->
            # ST-pair transitions chain at the PE issue floor (~215ns) while
            # mixed PV<->ST transitions each cost ~+100ns of array turnaround
            T_END = NQG * NKC + LAG
            for t0 in range(0, T_END, 3):
                batch = range(t0, min(t0 + 3, T_END))
                for t in batch:
                    if t < NQG * NKC:
                        emit_st_exp(t // NKC, t % NKC, t)
                for t in batch:
                    g, kc = t // NKC, t % NKC
                    for p in pv_at.get(t, ()):
                        emit_pv(p)
                        if p % NKC == NKC - 1:  # group fully summed
                            emit_drain(p // NKC)
                    if t == 2:
                        nc.vector.tensor_copy(wob[:], wo32[:])
                    if t == 20:
                        bps = stp.tile([128, C], F32, tag="st", bufs=3,
                                       name="bps")
                        nc.tensor.matmul(bps[:], ones1[:], bo32[:],
                                         start=True, stop=True)
                        nc.vector.tensor_copy(bias_bc[:], bps[:])
                    if t < NQG * NKC and g == 0 and t < 28:
                        emit_proj_piece(1 + t // 4, t % 4)
                    if t < NQG * NKC and g >= 1 and kc == 12:
                        emit_finish_a(g - 1)
            emit_finish_a(NQG - 1)

    nc.compile()
    return nc


def _prep_in_maps(x, w_qkv, w_out, b_out):
    x = np.asarray(x, dtype=np.float32).reshape(B, NPIX, C)
    w_qkv = np.asarray(w_qkv, dtype=np.float32)
    w_out = np.asarray(w_out, dtype=np.float32)
    b_out = np.asarray(b_out, dtype=np.float32)

    wq = np.ascontiguousarray(
        np.concatenate([w_qkv[:, 0:64], w_qkv[:, 192:256]], axis=1))
    wk = np.ascontiguousarray(
        np.concatenate([w_qkv[:, 64:128], w_qkv[:, 256:320]], axis=1))
    wv = np.ascontiguousarray(
        np.concatenate([w_qkv[:, 128:192], w_qkv[:, 320:384]], axis=1))
    wo = np.ascontiguousarray(w_out)
    bo = np.ascontiguousarray(b_out.reshape(1, C))

    in_maps = []
    for core in range(N_CORES):
        b, qh = core // 2, core % 2
        xbT = x[b].T                     # [C, NPIX]
        q0 = qh * NQ
        xt = np.ascontiguousarray(
            np.concatenate([xbT[:, q0:], xbT[:, :q0]], axis=1))
        in_maps.append({"xt": xt, "wq": wq, "wk": wk, "wv": wv,
                        "wo": wo, "bo": bo})
    return in_maps


def run(x, w_qkv, w_out, b_out, trace=False, **run_kwargs):
    if "nc" not in _CACHE:
        _CACHE["nc"] = _build_nc()
    nc = _CACHE["nc"]
    in_maps = _prep_in_maps(x, w_qkv, w_out, b_out)
    res = run_bass_kernel_spmd(nc, in_maps, core_ids=list(range(N_CORES)),
                               trace=trace, **run_kwargs)
    out = np.empty((B, NPIX, C), dtype=np.float32)
    for core in range(N_CORES):
        b, qh = core // 2, core % 2
        out[b, qh * NQ:(qh + 1) * NQ, :] = res.results[core]["out"]
    return out.reshape(B, 64, 64, C), res


def kernel(x, w_qkv, w_out, b_out):
    out, _ = run(x, w_qkv, w_out, b_out, trace=False)
    return out


# revision 23
# speedup vs baseline: 1.0168x; 1.0165x over previous
"""Trainium2 Bass kernel for nn_AttentionConv (B=4,H=W=64,C=128,heads=2).

Sharding: 8 cores = (batch b in 0..3) x (query-half qh in 0..1).
Each core computes full attention for its 2048 query pixels of batch b,
over all 4096 keys, both heads, plus the qkv and output 1x1-conv
projections.  No cross-core communication.

Host-side layout prep (part of sharding):
 - x[b] is transposed to [C, N] and rotated so this core's queries are
   columns 0..2047 (key order is softmax-invariant).
 - w_qkv is split into wq/wk/wv, each [128, 128] = both heads' 64 dims
   column-concatenated.

Per-core device algorithm (matmuls bf16, accumulate f32):
 - QT = wq^T x^T  -> [128(2h x 64d), 2048];  KT = wk^T x^T -> [128, 4096]
 - V natural [4096, 128] per key-chunk as lhsT tiles [128, 2, 128]:
   cols 0..63 = v_head, cols 64..127 = 1.0 (ones columns make the PV
   matmul also emit the softmax row-sum Z on partitions 64..127).
 - per query-group (512) x key-chunk (128): S^T = K Q^T in PSUM (both
   heads packed: disjoint PE row-groups + PSUM banks, run concurrently).
 - exp is SPLIT across two engines: ~2/3 of key chunks on ScalarE
   (ACT exp, scale=C^-0.5 folded in) and ~1/3 on VectorE via a
   Schraudolph-style bit trick: one tensor_scalar computes
   round(s*A + B) into int16 whose bits ARE the bf16 exp(s*SCALE)
   (A = SCALE*128/ln2, B = 128*(127-c)).  DVE f32->i16 converts RNE
   (hardware-probed); rel-err impact measured at <2e-3 end to end.
 - PV accumulates O' and Z in one PSUM tile per group (ones trick);
   normalization is commuted past the output projection: Z^T via K=1
   matmuls (the matmul transposes a row to a column), one tiny [128,8]
   reciprocal, out = (O'^T w_out) * (1/Z) + b via per-partition-scalar
   scalar_tensor_tensor combines.

PSUM budget (16KB/partition, exact): st pool 3 x [128,1024] f32 (12KB,
shared rotation also carrying projection/outproj/bias psum tiles) +
o_ps 1 x [128,1024] f32 (4KB, heads packed on free dim, single buffer
-- the epilogue drains it before the next group's first PV, which is
emitted 10 chunks late to guarantee the WAR gap without stalling PE).
"""

import math
import numpy as np

import concourse.bass as bass
import concourse.tile as tile
from concourse.tile import add_dep_helper
from concourse import bacc, mybir
from concourse.bass_utils import run_bass_kernel_spmd

F32 = mybir.dt.float32
BF16 = mybir.dt.bfloat16
I16 = mybir.dt.int16

B = 4
C = 128
NPIX = 4096          # 64*64 pixels per batch
NQ = 2048            # queries per core (half batch)
HC = 64              # head dim
KC = 128             # key chunk
NKC = NPIX // KC     # 32
QG = 512             # query group (per head; ST tile packs both heads)
NQG = NQ // QG       # 4
SCALE = float(C) ** -0.5
N_CORES = 8

# Schraudolph bf16-bit exp constants (DVE path)
C_MAGIC = 0.045
A_MAGIC = SCALE * 128.0 / math.log(2.0)
B_MAGIC = 128.0 * (127.0 - C_MAGIC)
# key chunks computed on DVE instead of ACT (per group)
DVE_KC = frozenset({8, 10, 18, 20, 22, 24, 26, 28, 30, 31})
DVE_KC_LAST = DVE_KC

LAG = 6              # PV emission lags ST/exp by this many chunks
FIRST_PV = 10        # earliest within-group step for a group's first PV

_CACHE = {}


def _build_nc():
    nc = bacc.Bacc("TRN2", target_bir_lowering=False, debug=False)

    xt_d = nc.dram_tensor("xt", [C, NPIX], F32, kind="ExternalInput")
    wq_d = nc.dram_tensor("wq", [C, 128], F32, kind="ExternalInput")
    wk_d = nc.dram_tensor("wk", [C, 128], F32, kind="ExternalInput")
    wv_d = nc.dram_tensor("wv", [C, 128], F32, kind="ExternalInput")
    wo_d = nc.dram_tensor("wo", [C, C], F32, kind="ExternalInput")
    bo_d = nc.dram_tensor("bo", [1, C], F32, kind="ExternalInput")
    out_d = nc.dram_tensor("out", [NQ, C], F32, kind="ExternalOutput")

    Exp = mybir.ActivationFunctionType.Exp

    with tile.TileContext(nc) as tc:
        with (
            tc.tile_pool(name="const", bufs=1) as const,
            tc.tile_pool(name="stage", bufs=4) as stage,
            tc.tile_pool(name="et", bufs=12) as etp,
            tc.tile_pool(name="rz", bufs=2) as rzp,
            tc.tile_pool(name="osb", bufs=2) as osbp,
            tc.tile_pool(name="st", bufs=1, space="PSUM") as stp,
            tc.tile_pool(name="op", bufs=1, space="PSUM") as opp,
        ):
            # ---- persistent SBUF tensors
            xtb = [const.tile([C, 512], BF16, tag=f"xtb{j}", name=f"xtb{j}")
                   for j in range(8)]
            qt = [const.tile([128, 512], BF16, tag=f"qt{j}", name=f"qt{j}")
                  for j in range(4)]
            kt = [const.tile([128, 512], BF16, tag=f"kt{j}", name=f"kt{j}")
                  for j in range(8)]
            v4 = [const.tile([128, 2, 128], BF16, tag=f"v4_{k}",
                             name=f"v4_{k}") for k in range(NKC)]
            rt = const.tile([128, NQ], BF16, tag="rt")
            bias_bc = const.tile([128, C], F32, tag="bias_bc")
            wqb = const.tile([C, 128], BF16, tag="wqb")
            wkb = const.tile([C, 128], BF16, tag="wkb")
            wvb = const.tile([C, 128], BF16, tag="wvb")
            wob = const.tile([C, C], BF16, tag="wob")
            ones1 = const.tile([1, C], F32, tag="ones1")
            ktf = const.tile([128, 128], BF16, tag="ktf")
            onesb = const.tile([1, C], BF16, tag="onesb")
            warm = const.tile([1, 2], F32, tag="warm")

            # dummy exp first: loads the ACT table set off the critical path
            nc.vector.memset(warm[:], 0.0)
            nc.scalar.activation(warm[:], warm[:], Exp)

            # PE warm-up while DMAs run (HAM clock-gate to K=8/8)
            junk = const.tile([C, 512], BF16, tag="junk")
            nc.vector.memset(junk[:], 0.25)
            wst = stp.tile([128, 2 * QG], F32, tag="st", bufs=3, name="warm_st")
            for w in range(6):
                nc.tensor.matmul(wst[:, 0:512], junk[:, 0:128], junk[:],
                                 start=True, stop=True)

            # DMAs: first x chunk, then weights needed early, then the rest
            xs = [stage.tile([C, 512], F32, tag=f"xs{j}", name=f"xs{j}")
                  for j in range(8)]
            nc.sync.dma_start(xs[0][:], xt_d[:, 0:512])
            for name, dram, dst in (("wq", wq_d, wqb), ("wk", wk_d, wkb),
                                    ("wv", wv_d, wvb)):
                w32 = stage.tile([C, 128], F32, tag="w32", name=f"w32_{name}")
                nc.sync.dma_start(w32[:], dram[:])
                nc.vector.tensor_copy(dst[:], w32[:])
            bo32 = stage.tile([1, C], F32, tag="bo32")
            nc.sync.dma_start(bo32[:], bo_d[:])
            wo32 = stage.tile([C, C], F32, tag="wo32", name="wo32")
            nc.sync.dma_start(wo32[:], wo_d[:])
            for j in range(1, 8):
                nc.sync.dma_start(xs[j][:], xt_d[:, j * 512:(j + 1) * 512])

            def emit_proj_piece(j, piece):
                # projections for one 512-pixel chunk, split into 4 pieces
                # emitted on consecutive steps so PE/DVE stay smooth
                if piece == 0:
                    nc.vector.tensor_copy(xtb[j][:], xs[j][:])
                    if j < 4:  # QT over local queries
                        p = stp.tile([128, 512], F32, tag="st", bufs=3,
                                     name=f"pq{j}")
                        nc.tensor.matmul(p[:], wqb[:], xtb[j][:],
                                         start=True, stop=True)
                        nc.vector.tensor_copy(qt[j][:], p[:])
                    return
                if piece == 1:
                    if j == 0:  # fast path: keys 0..127 first
                        pf = stp.tile([128, 128], F32, tag="st", bufs=3,
                                      name="pkf")
                        nc.tensor.matmul(pf[:], wkb[:], xtb[0][:, 0:128],
                                         start=True, stop=True)
                        nc.vector.tensor_copy(ktf[:], pf[:])
                    p = stp.tile([128, 512], F32, tag="st", bufs=3,
                                 name=f"pk{j}")
                    nc.tensor.matmul(p[:], wkb[:], xtb[j][:],
                                     start=True, stop=True)
                    nc.vector.tensor_copy(kt[j][:], p[:])
                    return
                for kq in (0, 1) if piece == 2 else (2, 3):
                    k = j * 4 + kq
                    pv = stp.tile([128, 128], F32, tag="st", bufs=3,
                                  name=f"pvj{k}")
                    nc.tensor.matmul(pv[:],
                                     xtb[j][:, kq * 128:(kq + 1) * 128],
                                     wvb[:], start=True, stop=True)
                    nc.vector.memset(v4[k][:, :, 64:128], 1.0)
                    nc.vector.tensor_copy(
                        v4[k][:, :, 0:64],
                        pv[:].rearrange("p (h d) -> p h d", h=2))

            def emit_proj_chunk(j):
                for piece in range(4):
                    emit_proj_piece(j, piece)

            emit_proj_chunk(0)
            nc.vector.memset(ones1[:], 1.0)
            nc.vector.memset(onesb[:], 1.0)

            # ---- attention ----
            o_ps = {}     # group -> PSUM accumulation tile
            ets = {}      # global chunk -> et tile

            zrow = {}

            def emit_drain(g):
                # the Z row first (it gates the Z-transpose matmuls)
                zr = rzp.tile([1, 2 * QG], BF16, tag="zrow", name=f"zrow_{g}")
                nc.vector.tensor_copy(zr[:], o_ps[g][64:65, :])
                zrow[g] = zr
                emit_drain_rt(g, 0)
                emit_drain_rt(g, 1)

            def emit_drain_rt(g, h):
                q0 = g * QG
                nc.vector.tensor_copy(
                    rt[h * HC:(h + 1) * HC, q0:q0 + QG],
                    o_ps[g][0:64, h * QG:(h + 1) * QG])

            fin = {}

            def emit_finish_a(g):
                # Z^T via K=1 matmuls (the matmul transposes a row into a
                # column), one [128,8] recip, then the outproj matmuls
                q0 = g * QG
                zt = stp.tile([128, 8], F32, tag="st", bufs=3,
                              name=f"zt_{g}")
                for j in range(8):
                    nc.tensor.matmul(zt[:, j:j + 1],
                                     zrow[g][0:1, j * 128:(j + 1) * 128],
                                     onesb[0:1, 0:1], start=True, stop=True)
                rz8 = rzp.tile([128, 8], F32, tag="rz8", name=f"rz8_{g}")
                nc.vector.reciprocal(rz8[:], zt[:])
                gp = stp.tile([128, 2 * QG], F32, tag="st", bufs=3,
                              name=f"gps_{g}")
                for i in range(4):
                    for h in range(2):
                        nc.tensor.matmul(
                            gp[:, h * QG + i * 128:h * QG + (i + 1) * 128],
                            rt[h * HC:(h + 1) * HC,
                               q0 + i * 128:q0 + (i + 1) * 128],
                            wob[h * HC:(h + 1) * HC, :],
                            start=True, stop=True)
                ob = osbp.tile([128, 512], F32, tag="osb", name=f"ob_{g}")
                last = g == NQG - 1
                for i in range(4):   # independent first halves (h0 + bias)
                    nc.vector.scalar_tensor_tensor(
                        ob[:, i * 128:(i + 1) * 128],
                        gp[:, i * 128:(i + 1) * 128],
                        rz8[:, i:i + 1], bias_bc[:],
                        mybir.AluOpType.mult, mybir.AluOpType.add)
                for i in range(4):   # second halves, chain distance 4 ops
                    obi = ob[:, i * 128:(i + 1) * 128]
                    nc.vector.scalar_tensor_tensor(
                        obi, gp[:, QG + i * 128:QG + (i + 1) * 128],
                        rz8[:, 4 + i:5 + i], obi,
                        mybir.AluOpType.mult, mybir.AluOpType.add)
                    if last:    # per-block DMA so the tail drains early,
                        # alternating issue queues to overlap descriptor gen
                        eng = nc.sync if i % 2 == 0 else nc.gpsimd
                        eng.dma_start(
                            out_d[q0 + i * 128:q0 + (i + 1) * 128, :], obi)
                if not last:
                    nc.sync.dma_start(
                        out_d[q0:q0 + QG, :].rearrange(
                            "(c r) w -> r c w", r=128),
                        ob[:].rearrange("p (c w) -> p c w", w=128))

            def emit_st_exp(g, kc, t):
                st = stp.tile([128, 2 * QG], F32, tag="st", bufs=3,
                              name=f"st_{g}_{kc}")
                if g == 0 and kc == 0:
                    ktt, ks = ktf, slice(0, 128)
                else:
                    ktt = kt[kc // 4]
                    ks = slice((kc % 4) * 128, (kc % 4 + 1) * 128)
                for h in range(2):
                    hp = slice(h * HC, (h + 1) * HC)
                    nc.tensor.matmul(
                        st[:, h * QG:(h + 1) * QG],
                        ktt[hp, ks], qt[g][hp, :],
                        start=True, stop=True)
                et = etp.tile([128, 2 * QG], BF16, tag="et",
                              name=f"et_{g}_{kc}")
                if kc in (DVE_KC_LAST if g == NQG - 1 else DVE_KC):
                    nc.vector.tensor_scalar(
                        et[:].bitcast(I16), st[:],
                        float(A_MAGIC), float(B_MAGIC),
                        mybir.AluOpType.mult, mybir.AluOpType.add)
                else:
                    nc.scalar.activation(et[:], st[:], Exp, scale=SCALE)
                ets[t] = et

            def emit_pv(p):
                g, pk = p // NKC, p % NKC
                if p % NKC == 0:
                    o_ps[g] = opp.tile([128, 2 * QG], F32, tag="ops",
                                       name=f"o_ps_{g}")
                et = ets.pop(p)
                for h in range(2):
                    nc.tensor.matmul(
                        o_ps[g][:, h * QG:(h + 1) * QG], v4[pk][:, h, :],
                        et[:, h * QG:(h + 1) * QG],
                        start=(pk == 0), stop=(pk == NKC - 1))

            # PV(p) emission step: lag LAG behind ST, but never before
            # within-group step FIRST_PV (o_ps single-buffer WAR gap)
            pv_at = {}
            for p in range(NQG * NKC):
                t = max(p + LAG, (p // NKC) * NKC + FIRST_PV)
                pv_at.setdefault(t, []).append(p)

            # STs are emitted in batches of 3 then their PVs: ST-pair ->
            # ST-pair transitions chain at the PE issue floor (~215ns) while
            # mixed PV<->ST transitions each cost ~+100ns of array turnaround
            T_END = NQG * NKC + LAG
            for t0 in range(0, T_END, 3):
                batch = range(t0, min(t0 + 3, T_END))
                for t in batch:
                    if t < NQG * NKC:
                        emit_st_exp(t // NKC, t % NKC, t)
                for t in batch:
                    g, kc = t // NKC, t % NKC
                    for p in pv_at.get(t, ()):
                        emit_pv(p)
                        if p % NKC == NKC - 1:  # group fully summed
                            emit_drain(p // NKC)
                    if t == 30:
                        nc.vector.tensor_copy(wob[:], wo32[:])
                    if t == 34:
                        bps = stp.tile([128, C], F32, tag="st", bufs=3,
                                       name="bps")
                        nc.tensor.matmul(bps[:], ones1[:], bo32[:],
                                         start=True, stop=True)
                        nc.vector.tensor_copy(bias_bc[:], bps[:])
                    if t < NQG * NKC and g == 0 and t < 28:
                        emit_proj_piece(1 + t // 4, t % 4)
                    if t < NQG * NKC and g >= 1 and kc == 12:
                        emit_finish_a(g - 1)
            emit_finish_a(NQG - 1)

    nc.compile()
    return nc


def _prep_in_maps(x, w_qkv, w_out, b_out):
    x = np.asarray(x, dtype=np.float32).reshape(B, NPIX, C)
    w_qkv = np.asarray(w_qkv, dtype=np.float32)
    w_out = np.asarray(w_out, dtype=np.float32)
    b_out = np.asarray(b_out, dtype=np.float32)

    wq = np.ascontiguousarray(
        np.concatenate([w_qkv[:, 0:64], w_qkv[:, 192:256]], axis=1))
    wk = np.ascontiguousarray(
        np.concatenate([w_qkv[:, 64:128], w_qkv[:, 256:320]], axis=1))
    wv = np.ascontiguousarray(
        np.concatenate([w_qkv[:, 128:192], w_qkv[:, 320:384]], axis=1))
    wo = np.ascontiguousarray(w_out)
    bo = np.ascontiguousarray(b_out.reshape(1, C))

    in_maps = []
    for core in range(N_CORES):
        b, qh = core // 2, core % 2
        xbT = x[b].T                     # [C, NPIX]
        q0 = qh * NQ
        xt = np.ascontiguousarray(
            np.concatenate([xbT[:, q0:], xbT[:, :q0]], axis=1))
        in_maps.append({"xt": xt, "wq": wq, "wk": wk, "wv": wv,
                        "wo": wo, "bo": bo})
    return in_maps


def run(x, w_qkv, w_out, b_out, trace=False, **run_kwargs):
    if "nc" not in _CACHE:
        _CACHE["nc"] = _build_nc()
    nc = _CACHE["nc"]
    in_maps = _prep_in_maps(x, w_qkv, w_out, b_out)
    res = run_bass_kernel_spmd(nc, in_maps, core_ids=list(range(N_CORES)),
                               trace=trace, **run_kwargs)
    out = np.empty((B, NPIX, C), dtype=np.float32)
    for core in range(N_CORES):
        b, qh = core // 2, core % 2
        out[b, qh * NQ:(qh + 1) * NQ, :] = res.results[core]["out"]
    return out.reshape(B, 64, 64, C), res


def kernel(x, w_qkv, w_out, b_out):
    out, _ = run(x, w_qkv, w_out, b_out, trace=False)
    return out


# revision 24
# speedup vs baseline: 1.0291x; 1.0120x over previous
"""Trainium2 Bass kernel for nn_AttentionConv (B=4,H=W=64,C=128,heads=2).

Sharding: 8 cores = (batch b in 0..3) x (query-half qh in 0..1).
Each core computes full attention for its 2048 query pixels of batch b,
over all 4096 keys, both heads, plus the qkv and output 1x1-conv
projections.  No cross-core communication.

Host-side layout prep (part of sharding):
 - x[b] is transposed to [C, N] and rotated so this core's queries are
   columns 0..2047 (key order is softmax-invariant).
 - w_qkv is split into wq/wk/wv, each [128, 128] = both heads' 64 dims
   column-concatenated.

Per-core device algorithm (matmuls bf16, accumulate f32):
 - QT = wq^T x^T  -> [128(2h x 64d), 2048];  KT = wk^T x^T -> [128, 4096]
 - V natural [4096, 128] per key-chunk as lhsT tiles [128, 2, 128]:
   cols 0..63 = v_head, cols 64..127 = 1.0 (ones columns make the PV
   matmul also emit the softmax row-sum Z on partitions 64..127).
 - per query-group (512) x key-chunk (128): S^T = K Q^T in PSUM (both
   heads packed: disjoint PE row-groups + PSUM banks, run concurrently).
 - exp is SPLIT across two engines: ~2/3 of key chunks on ScalarE
   (ACT exp, scale=C^-0.5 folded in) and ~1/3 on VectorE via a
   Schraudolph-style bit trick: one tensor_scalar computes
   round(s*A + B) into int16 whose bits ARE the bf16 exp(s*SCALE)
   (A = SCALE*128/ln2, B = 128*(127-c)).  DVE f32->i16 converts RNE
   (hardware-probed); rel-err impact measured at <2e-3 end to end.
 - PV accumulates O' and Z in one PSUM tile per group (ones trick);
   normalization is commuted past the output projection: Z^T via K=1
   matmuls (the matmul transposes a row to a column), one tiny [128,8]
   reciprocal, out = (O'^T w_out) * (1/Z) + b via per-partition-scalar
   scalar_tensor_tensor combines.

PSUM budget (16KB/partition, exact): st pool 3 x [128,1024] f32 (12KB,
shared rotation also carrying projection/outproj/bias psum tiles) +
o_ps 1 x [128,1024] f32 (4KB, heads packed on free dim, single buffer
-- the epilogue drains it before the next group's first PV, which is
emitted 10 chunks late to guarantee the WAR gap without stalling PE).
"""

import math
import numpy as np

import concourse.bass as bass
import concourse.tile as tile
from concourse.tile import add_dep_helper
from concourse import bacc, mybir
from concourse.bass_utils import run_bass_kernel_spmd

F32 = mybir.dt.float32
BF16 = mybir.dt.bfloat16
I16 = mybir.dt.int16

B = 4
C = 128
NPIX = 4096          # 64*64 pixels per batch
NQ = 2048            # queries per core (half batch)
HC = 64              # head dim
KC = 128             # key chunk
NKC = NPIX // KC     # 32
QG = 512             # query group (per head; ST tile packs both heads)
NQG = NQ // QG       # 4
SCALE = float(C) ** -0.5
N_CORES = 8

# Schraudolph bf16-bit exp constants (DVE path)
C_MAGIC = 0.045
A_MAGIC = SCALE * 128.0 / math.log(2.0)
B_MAGIC = 128.0 * (127.0 - C_MAGIC)
# key chunks computed on DVE instead of ACT (per group)
DVE_KC = frozenset({8, 10, 18, 20, 22, 24, 26, 28, 30, 31})
DVE_KC_LAST = DVE_KC

LAG = 6              # PV emission lags ST/exp by this many chunks
FIRST_PV = 10        # earliest within-group step for a group's first PV

_CACHE = {}


def _build_nc():
    nc = bacc.Bacc("TRN2", target_bir_lowering=False, debug=False)

    xt_d = nc.dram_tensor("xt", [C, NPIX], F32, kind="ExternalInput")
    wq_d = nc.dram_tensor("wq", [C, 128], F32, kind="ExternalInput")
    wk_d = nc.dram_tensor("wk", [C, 128], F32, kind="ExternalInput")
    wv_d = nc.dram_tensor("wv", [C, 128], F32, kind="ExternalInput")
    wo_d = nc.dram_tensor("wo", [C, C], F32, kind="ExternalInput")
    bo_d = nc.dram_tensor("bo", [1, C], F32, kind="ExternalInput")
    out_d = nc.dram_tensor("out", [NQ, C], F32, kind="ExternalOutput")

    Exp = mybir.ActivationFunctionType.Exp

    with tile.TileContext(nc) as tc:
        with (
            tc.tile_pool(name="const", bufs=1) as const,
            tc.tile_pool(name="stage", bufs=4) as stage,
            tc.tile_pool(name="et", bufs=16) as etp,
            tc.tile_pool(name="rz", bufs=2) as rzp,
            tc.tile_pool(name="osb", bufs=2) as osbp,
            tc.tile_pool(name="st", bufs=1, space="PSUM") as stp,
            tc.tile_pool(name="op", bufs=1, space="PSUM") as opp,
        ):
            # ---- persistent SBUF tensors
            xtb = [const.tile([C, 512], BF16, tag=f"xtb{j}", name=f"xtb{j}")
                   for j in range(8)]
            qt = [const.tile([128, 512], BF16, tag=f"qt{j}", name=f"qt{j}")
                  for j in range(4)]
            kt = [const.tile([128, 512], BF16, tag=f"kt{j}", name=f"kt{j}")
                  for j in range(8)]
            v4 = [const.tile([128, 2, 128], BF16, tag=f"v4_{k}",
                             name=f"v4_{k}") for k in range(NKC)]
            rt = const.tile([128, NQ], BF16, tag="rt")
            bias_bc = const.tile([128, C], F32, tag="bias_bc")
            wqb = const.tile([C, 128], BF16, tag="wqb")
            wkb = const.tile([C, 128], BF16, tag="wkb")
            wvb = const.tile([C, 128], BF16, tag="wvb")
            wob = const.tile([C, C], BF16, tag="wob")
            ones1 = const.tile([1, C], F32, tag="ones1")
            ktf = const.tile([128, 128], BF16, tag="ktf")
            onesb = const.tile([1, C], BF16, tag="onesb")
            warm = const.tile([1, 2], F32, tag="warm")

            # dummy exp first: loads the ACT table set off the critical path
            nc.vector.memset(warm[:], 0.0)
            nc.scalar.activation(warm[:], warm[:], Exp)

            # PE warm-up while DMAs run (HAM clock-gate to K=8/8)
            junk = const.tile([C, 512], BF16, tag="junk")
            nc.vector.memset(junk[:], 0.25)
            wst = stp.tile([128, 2 * QG], F32, tag="st", bufs=3, name="warm_st")
            for w in range(6):
                nc.tensor.matmul(wst[:, 0:512], junk[:, 0:128], junk[:],
                                 start=True, stop=True)

            # DMAs: first x chunk, then weights needed early, then the rest
            xs = [stage.tile([C, 512], F32, tag=f"xs{j}", name=f"xs{j}")
                  for j in range(8)]
            nc.sync.dma_start(xs[0][:], xt_d[:, 0:512])
            for name, dram, dst in (("wq", wq_d, wqb), ("wk", wk_d, wkb),
                                    ("wv", wv_d, wvb)):
                w32 = stage.tile([C, 128], F32, tag="w32", name=f"w32_{name}")
                nc.sync.dma_start(w32[:], dram[:])
                nc.vector.tensor_copy(dst[:], w32[:])
            bo32 = stage.tile([1, C], F32, tag="bo32")
            nc.sync.dma_start(bo32[:], bo_d[:])
            wo32 = stage.tile([C, C], F32, tag="wo32", name="wo32")
            nc.sync.dma_start(wo32[:], wo_d[:])
            for j in range(1, 8):
                nc.sync.dma_start(xs[j][:], xt_d[:, j * 512:(j + 1) * 512])

            def emit_proj_piece(j, piece):
                # projections for one 512-pixel chunk, split into 4 pieces
                # emitted on consecutive steps so PE/DVE stay smooth
                if piece == 0:
                    nc.vector.tensor_copy(xtb[j][:], xs[j][:])
                    if j < 4:  # QT over local queries
                        p = stp.tile([128, 512], F32, tag="st", bufs=3,
                                     name=f"pq{j}")
                        nc.tensor.matmul(p[:], wqb[:], xtb[j][:],
                                         start=True, stop=True)
                        nc.vector.tensor_copy(qt[j][:], p[:])
                    return
                if piece == 1:
                    if j == 0:  # fast path: keys 0..127 first
                        pf = stp.tile([128, 128], F32, tag="st", bufs=3,
                                      name="pkf")
                        nc.tensor.matmul(pf[:], wkb[:], xtb[0][:, 0:128],
                                         start=True, stop=True)
                        nc.vector.tensor_copy(ktf[:], pf[:])
                    p = stp.tile([128, 512], F32, tag="st", bufs=3,
                                 name=f"pk{j}")
                    nc.tensor.matmul(p[:], wkb[:], xtb[j][:],
                                     start=True, stop=True)
                    nc.vector.tensor_copy(kt[j][:], p[:])
                    return
                for kq in (0, 1) if piece == 2 else (2, 3):
                    k = j * 4 + kq
                    pv = stp.tile([128, 128], F32, tag="st", bufs=3,
                                  name=f"pvj{k}")
                    nc.tensor.matmul(pv[:],
                                     xtb[j][:, kq * 128:(kq + 1) * 128],
                                     wvb[:], start=True, stop=True)
                    nc.vector.memset(v4[k][:, :, 64:128], 1.0)
                    nc.vector.tensor_copy(
                        v4[k][:, :, 0:64],
                        pv[:].rearrange("p (h d) -> p h d", h=2))

            def emit_proj_chunk(j):
                for piece in range(4):
                    emit_proj_piece(j, piece)

            emit_proj_chunk(0)
            nc.vector.memset(ones1[:], 1.0)
            nc.vector.memset(onesb[:], 1.0)

            # ---- attention ----
            o_ps = {}     # group -> PSUM accumulation tile
            ets = {}      # global chunk -> et tile

            zrow = {}

            def emit_drain(g):
                # the Z row first (it gates the Z-transpose matmuls)
                zr = rzp.tile([1, 2 * QG], BF16, tag="zrow", name=f"zrow_{g}")
                nc.vector.tensor_copy(zr[:], o_ps[g][64:65, :])
                zrow[g] = zr
                emit_drain_rt(g, 0)
                emit_drain_rt(g, 1)

            def emit_drain_rt(g, h):
                q0 = g * QG
                nc.vector.tensor_copy(
                    rt[h * HC:(h + 1) * HC, q0:q0 + QG],
                    o_ps[g][0:64, h * QG:(h + 1) * QG])

            fin = {}

            def emit_finish_a(g):
                # Z^T via K=1 matmuls (the matmul transposes a row into a
                # column), one [128,8] recip, then the outproj matmuls
                q0 = g * QG
                zt = stp.tile([128, 8], F32, tag="st", bufs=3,
                              name=f"zt_{g}")
                for j in range(8):
                    nc.tensor.matmul(zt[:, j:j + 1],
                                     zrow[g][0:1, j * 128:(j + 1) * 128],
                                     onesb[0:1, 0:1], start=True, stop=True)
                rz8 = rzp.tile([128, 8], F32, tag="rz8", name=f"rz8_{g}")
                nc.vector.reciprocal(rz8[:], zt[:])
                gp = stp.tile([128, 2 * QG], F32, tag="st", bufs=3,
                              name=f"gps_{g}")
                for i in range(4):
                    for h in range(2):
                        nc.tensor.matmul(
                            gp[:, h * QG + i * 128:h * QG + (i + 1) * 128],
                            rt[h * HC:(h + 1) * HC,
                               q0 + i * 128:q0 + (i + 1) * 128],
                            wob[h * HC:(h + 1) * HC, :],
                            start=True, stop=True)
                ob = osbp.tile([128, 512], F32, tag="osb", name=f"ob_{g}")
                last = g == NQG - 1
                for i in range(4):   # independent first halves (h0 + bias)
                    nc.vector.scalar_tensor_tensor(
                        ob[:, i * 128:(i + 1) * 128],
                        gp[:, i * 128:(i + 1) * 128],
                        rz8[:, i:i + 1], bias_bc[:],
                        mybir.AluOpType.mult, mybir.AluOpType.add)
                for i in range(4):   # second halves, chain distance 4 ops
                    obi = ob[:, i * 128:(i + 1) * 128]
                    nc.vector.scalar_tensor_tensor(
                        obi, gp[:, QG + i * 128:QG + (i + 1) * 128],
                        rz8[:, 4 + i:5 + i], obi,
                        mybir.AluOpType.mult, mybir.AluOpType.add)
                    if last:    # per-block DMA so the tail drains early,
                        # alternating issue queues to overlap descriptor gen
                        eng = nc.sync if i % 2 == 0 else nc.gpsimd
                        eng.dma_start(
                            out_d[q0 + i * 128:q0 + (i + 1) * 128, :], obi)
                if not last:
                    nc.sync.dma_start(
                        out_d[q0:q0 + QG, :].rearrange(
                            "(c r) w -> r c w", r=128),
                        ob[:].rearrange("p (c w) -> p c w", w=128))

            def emit_st_exp(g, kc, t):
                st = stp.tile([128, 2 * QG], F32, tag="st", bufs=3,
                              name=f"st_{g}_{kc}")
                if g == 0 and kc == 0:
                    ktt, ks = ktf, slice(0, 128)
                else:
                    ktt = kt[kc // 4]
                    ks = slice((kc % 4) * 128, (kc % 4 + 1) * 128)
                for h in range(2):
                    hp = slice(h * HC, (h + 1) * HC)
                    nc.tensor.matmul(
                        st[:, h * QG:(h + 1) * QG],
                        ktt[hp, ks], qt[g][hp, :],
                        start=True, stop=True)
                et = etp.tile([128, 2 * QG], BF16, tag="et",
                              name=f"et_{g}_{kc}")
                if kc in (DVE_KC_LAST if g == NQG - 1 else DVE_KC):
                    nc.vector.tensor_scalar(
                        et[:].bitcast(I16), st[:],
                        float(A_MAGIC), float(B_MAGIC),
                        mybir.AluOpType.mult, mybir.AluOpType.add)
                else:
                    nc.scalar.activation(et[:], st[:], Exp, scale=SCALE)
                ets[t] = et

            def emit_pv(p):
                g, pk = p // NKC, p % NKC
                if p % NKC == 0:
                    o_ps[g] = opp.tile([128, 2 * QG], F32, tag="ops",
                                       name=f"o_ps_{g}")
                et = ets.pop(p)
                for h in range(2):
                    nc.tensor.matmul(
                        o_ps[g][:, h * QG:(h + 1) * QG], v4[pk][:, h, :],
                        et[:, h * QG:(h + 1) * QG],
                        start=(pk == 0), stop=(pk == NKC - 1))

            # PV(p) emission step: lag LAG behind ST, but never before
            # within-group step FIRST_PV (o_ps single-buffer WAR gap)
            pv_at = {}
            for p in range(NQG * NKC):
                t = max(p + LAG, (p // NKC) * NKC + FIRST_PV)
                pv_at.setdefault(t, []).append(p)

            # STs are emitted in batches of 3 then their PVs: ST-pair ->
            # ST-pair transitions chain at the PE issue floor (~215ns) while
            # mixed PV<->ST transitions each cost ~+100ns of array turnaround
            T_END = NQG * NKC + LAG
            for t0 in range(0, T_END, 3):
                batch = range(t0, min(t0 + 3, T_END))
                for t in batch:
                    if t < NQG * NKC:
                        emit_st_exp(t // NKC, t % NKC, t)
                for t in batch:
                    g, kc = t // NKC, t % NKC
                    for p in pv_at.get(t, ()):
                        emit_pv(p)
                        if p % NKC == NKC - 1:  # group fully summed
                            emit_drain(p // NKC)
                    if t == 30:
                        nc.vector.tensor_copy(wob[:], wo32[:])
                    if t == 34:
                        bps = stp.tile([128, C], F32, tag="st", bufs=3,
                                       name="bps")
                        nc.tensor.matmul(bps[:], ones1[:], bo32[:],
                                         start=True, stop=True)
                        nc.vector.tensor_copy(bias_bc[:], bps[:])
                    if t < NQG * NKC and g == 0 and t < 28:
                        emit_proj_piece(1 + t // 4, t % 4)
                    if t < NQG * NKC and g >= 1 and kc == 12:
                        emit_finish_a(g - 1)
            emit_finish_a(NQG - 1)

    nc.compile()
    return nc


def _prep_in_maps(x, w_qkv, w_out, b_out):
    x = np.asarray(x, dtype=np.float32).reshape(B, NPIX, C)
    w_qkv = np.asarray(w_qkv, dtype=np.float32)
    w_out = np.asarray(w_out, dtype=np.float32)
    b_out = np.asarray(b_out, dtype=np.float32)

    wq = np.ascontiguousarray(
        np.concatenate([w_qkv[:, 0:64], w_qkv[:, 192:256]], axis=1))
    wk = np.ascontiguousarray(
        np.concatenate([w_qkv[:, 64:128], w_qkv[:, 256:320]], axis=1))
    wv = np.ascontiguousarray(
        np.concatenate([w_qkv[:, 128:192], w_qkv[:, 320:384]], axis=1))
    wo = np.ascontiguousarray(w_out)
    bo = np.ascontiguousarray(b_out.reshape(1, C))

    in_maps = []
    for core in range(N_CORES):
        b, qh = core // 2, core % 2
        xbT = x[b].T                     # [C, NPIX]
        q0 = qh * NQ
        xt = np.ascontiguousarray(
            np.concatenate([xbT[:, q0:], xbT[:, :q0]], axis=1))
        in_maps.append({"xt": xt, "wq": wq, "wk": wk, "wv": wv,
                        "wo": wo, "bo": bo})
    return in_maps


def run(x, w_qkv, w_out, b_out, trace=False, **run_kwargs):
    if "nc" not in _CACHE:
        _CACHE["nc"] = _build_nc()
    nc = _CACHE["nc"]
    in_maps = _prep_in_maps(x, w_qkv, w_out, b_out)
    res = run_bass_kernel_spmd(nc, in_maps, core_ids=list(range(N_CORES)),
                               trace=trace, **run_kwargs)
    out = np.empty((B, NPIX, C), dtype=np.float32)
    for core in range(N_CORES):
        b, qh = core // 2, core % 2
        out[b, qh * NQ:(qh + 1) * NQ, :] = res.results[core]["out"]
    return out.reshape(B, 64, 64, C), res


def kernel(x, w_qkv, w_out, b_out):
    out, _ = run(x, w_qkv, w_out, b_out, trace=False)
    return out


# revision 25
# speedup vs baseline: 1.0314x; 1.0023x over previous
"""Trainium2 Bass kernel for nn_AttentionConv (B=4,H=W=64,C=128,heads=2).

Sharding: 8 cores = (batch b in 0..3) x (query-half qh in 0..1).
Each core computes full attention for its 2048 query pixels of batch b,
over all 4096 keys, both heads, plus the qkv and output 1x1-conv
projections.  No cross-core communication.

Host-side layout prep (part of sharding):
 - x[b] is transposed to [C, N] and rotated so this core's queries are
   columns 0..2047 (key order is softmax-invariant).
 - w_qkv is split into wq/wk/wv, each [128, 128] = both heads' 64 dims
   column-concatenated.

Per-core device algorithm (matmuls bf16, accumulate f32):
 - QT = wq^T x^T  -> [128(2h x 64d), 2048];  KT = wk^T x^T -> [128, 4096]
 - V natural [4096, 128] per key-chunk as lhsT tiles [128, 2, 128]:
   cols 0..63 = v_head, cols 64..127 = 1.0 (ones columns make the PV
   matmul also emit the softmax row-sum Z on partitions 64..127).
 - per query-group (512) x key-chunk (128): S^T = K Q^T in PSUM (both
   heads packed: disjoint PE row-groups + PSUM banks, run concurrently).
 - exp is SPLIT across two engines: ~2/3 of key chunks on ScalarE
   (ACT exp, scale=C^-0.5 folded in) and ~1/3 on VectorE via a
   Schraudolph-style bit trick: one tensor_scalar computes
   round(s*A + B) into int16 whose bits ARE the bf16 exp(s*SCALE)
   (A = SCALE*128/ln2, B = 128*(127-c)).  DVE f32->i16 converts RNE
   (hardware-probed); rel-err impact measured at <2e-3 end to end.
 - PV accumulates O' and Z in one PSUM tile per group (ones trick);
   normalization is commuted past the output projection: Z^T via K=1
   matmuls (the matmul transposes a row to a column), one tiny [128,8]
   reciprocal, out = (O'^T w_out) * (1/Z) + b via per-partition-scalar
   scalar_tensor_tensor combines.

PSUM budget (16KB/partition, exact): st pool 3 x [128,1024] f32 (12KB,
shared rotation also carrying projection/outproj/bias psum tiles) +
o_ps 1 x [128,1024] f32 (4KB, heads packed on free dim, single buffer
-- the epilogue drains it before the next group's first PV, which is
emitted 10 chunks late to guarantee the WAR gap without stalling PE).
"""

import math
import numpy as np

import concourse.bass as bass
import concourse.tile as tile
from concourse.tile import add_dep_helper
from concourse import bacc, mybir
from concourse.bass_utils import run_bass_kernel_spmd

F32 = mybir.dt.float32
BF16 = mybir.dt.bfloat16
I16 = mybir.dt.int16

B = 4
C = 128
NPIX = 4096          # 64*64 pixels per batch
NQ = 2048            # queries per core (half batch)
HC = 64              # head dim
KC = 128             # key chunk
NKC = NPIX // KC     # 32
QG = 512             # query group (per head; ST tile packs both heads)
NQG = NQ // QG       # 4
SCALE = float(C) ** -0.5
N_CORES = 8

# Schraudolph bf16-bit exp constants (DVE path)
C_MAGIC = 0.045
A_MAGIC = SCALE * 128.0 / math.log(2.0)
B_MAGIC = 128.0 * (127.0 - C_MAGIC)
# key chunks computed on DVE instead of ACT (per group)
DVE_KC = frozenset({8, 10, 18, 20, 22, 24, 26, 28, 30, 31})
DVE_KC_LAST = DVE_KC

LAG = 6              # PV emission lags ST/exp by this many chunks
FIRST_PV = 11        # earliest within-group step for a group's first PV

_CACHE = {}


def _build_nc():
    nc = bacc.Bacc("TRN2", target_bir_lowering=False, debug=False)

    xt_d = nc.dram_tensor("xt", [C, NPIX], F32, kind="ExternalInput")
    wq_d = nc.dram_tensor("wq", [C, 128], F32, kind="ExternalInput")
    wk_d = nc.dram_tensor("wk", [C, 128], F32, kind="ExternalInput")
    wv_d = nc.dram_tensor("wv", [C, 128], F32, kind="ExternalInput")
    wo_d = nc.dram_tensor("wo", [C, C], F32, kind="ExternalInput")
    bo_d = nc.dram_tensor("bo", [1, C], F32, kind="ExternalInput")
    out_d = nc.dram_tensor("out", [NQ, C], F32, kind="ExternalOutput")

    Exp = mybir.ActivationFunctionType.Exp

    with tile.TileContext(nc) as tc:
        with (
            tc.tile_pool(name="const", bufs=1) as const,
            tc.tile_pool(name="stage", bufs=4) as stage,
            tc.tile_pool(name="et", bufs=16) as etp,
            tc.tile_pool(name="rz", bufs=2) as rzp,
            tc.tile_pool(name="osb", bufs=2) as osbp,
            tc.tile_pool(name="st", bufs=1, space="PSUM") as stp,
            tc.tile_pool(name="op", bufs=1, space="PSUM") as opp,
        ):
            # ---- persistent SBUF tensors
            xtb = [const.tile([C, 512], BF16, tag=f"xtb{j}", name=f"xtb{j}")
                   for j in range(8)]
            qt = [const.tile([128, 512], BF16, tag=f"qt{j}", name=f"qt{j}")
                  for j in range(4)]
            kt = [const.tile([128, 512], BF16, tag=f"kt{j}", name=f"kt{j}")
                  for j in range(8)]
            v4 = [const.tile([128, 2, 128], BF16, tag=f"v4_{k}",
                             name=f"v4_{k}") for k in range(NKC)]
            rt = const.tile([128, NQ], BF16, tag="rt")
            bias_bc = const.tile([128, C], F32, tag="bias_bc")
            wqb = const.tile([C, 128], BF16, tag="wqb")
            wkb = const.tile([C, 128], BF16, tag="wkb")
            wvb = const.tile([C, 128], BF16, tag="wvb")
            wob = const.tile([C, C], BF16, tag="wob")
            ones1 = const.tile([1, C], F32, tag="ones1")
            ktf = const.tile([128, 128], BF16, tag="ktf")
            onesb = const.tile([1, C], BF16, tag="onesb")
            warm = const.tile([1, 2], F32, tag="warm")

            # dummy exp first: loads the ACT table set off the critical path
            nc.vector.memset(warm[:], 0.0)
            nc.scalar.activation(warm[:], warm[:], Exp)

            # PE warm-up while DMAs run (HAM clock-gate to K=8/8)
            junk = const.tile([C, 512], BF16, tag="junk")
            nc.vector.memset(junk[:], 0.25)
            wst = stp.tile([128, 2 * QG], F32, tag="st", bufs=3, name="warm_st")
            for w in range(6):
                nc.tensor.matmul(wst[:, 0:512], junk[:, 0:128], junk[:],
                                 start=True, stop=True)

            # DMAs: first x chunk, then weights needed early, then the rest
            xs = [stage.tile([C, 512], F32, tag=f"xs{j}", name=f"xs{j}")
                  for j in range(8)]
            nc.sync.dma_start(xs[0][:], xt_d[:, 0:512])
            for name, dram, dst in (("wq", wq_d, wqb), ("wk", wk_d, wkb),
                                    ("wv", wv_d, wvb)):
                w32 = stage.tile([C, 128], F32, tag="w32", name=f"w32_{name}")
                nc.sync.dma_start(w32[:], dram[:])
                nc.vector.tensor_copy(dst[:], w32[:])
            bo32 = stage.tile([1, C], F32, tag="bo32")
            nc.sync.dma_start(bo32[:], bo_d[:])
            wo32 = stage.tile([C, C], F32, tag="wo32", name="wo32")
            nc.sync.dma_start(wo32[:], wo_d[:])
            for j in range(1, 8):
                nc.sync.dma_start(xs[j][:], xt_d[:, j * 512:(j + 1) * 512])

            def emit_proj_piece(j, piece):
                # projections for one 512-pixel chunk, split into 4 pieces
                # emitted on consecutive steps so PE/DVE stay smooth
                if piece == 0:
                    nc.vector.tensor_copy(xtb[j][:], xs[j][:])
                    if j < 4:  # QT over local queries
                        p = stp.tile([128, 512], F32, tag="st", bufs=3,
                                     name=f"pq{j}")
                        nc.tensor.matmul(p[:], wqb[:], xtb[j][:],
                                         start=True, stop=True)
                        nc.vector.tensor_copy(qt[j][:], p[:])
                    return
                if piece == 1:
                    if j == 0:  # fast path: keys 0..127 first
                        pf = stp.tile([128, 128], F32, tag="st", bufs=3,
                                      name="pkf")
                        nc.tensor.matmul(pf[:], wkb[:], xtb[0][:, 0:128],
                                         start=True, stop=True)
                        nc.vector.tensor_copy(ktf[:], pf[:])
                    p = stp.tile([128, 512], F32, tag="st", bufs=3,
                                 name=f"pk{j}")
                    nc.tensor.matmul(p[:], wkb[:], xtb[j][:],
                                     start=True, stop=True)
                    nc.vector.tensor_copy(kt[j][:], p[:])
                    return
                for kq in (0, 1) if piece == 2 else (2, 3):
                    k = j * 4 + kq
                    pv = stp.tile([128, 128], F32, tag="st", bufs=3,
                                  name=f"pvj{k}")
                    nc.tensor.matmul(pv[:],
                                     xtb[j][:, kq * 128:(kq + 1) * 128],
                                     wvb[:], start=True, stop=True)
                    nc.vector.memset(v4[k][:, :, 64:128], 1.0)
                    nc.vector.tensor_copy(
                        v4[k][:, :, 0:64],
                        pv[:].rearrange("p (h d) -> p h d", h=2))

            def emit_proj_chunk(j):
                for piece in range(4):
                    emit_proj_piece(j, piece)

            emit_proj_chunk(0)
            nc.vector.memset(ones1[:], 1.0)
            nc.vector.memset(onesb[:], 1.0)

            # ---- attention ----
            o_ps = {}     # group -> PSUM accumulation tile
            ets = {}      # global chunk -> et tile

            zrow = {}

            def emit_drain(g):
                # the Z row first (it gates the Z-transpose matmuls)
                zr = rzp.tile([1, 2 * QG], BF16, tag="zrow", name=f"zrow_{g}")
                nc.vector.tensor_copy(zr[:], o_ps[g][64:65, :])
                zrow[g] = zr
                if g == NQG - 1:
                    emit_drain_rt(g, 0)
                    emit_drain_rt(g, 1)

            def emit_drain_rt(g, h):
                q0 = g * QG
                nc.vector.tensor_copy(
                    rt[h * HC:(h + 1) * HC, q0:q0 + QG],
                    o_ps[g][0:64, h * QG:(h + 1) * QG])

            fin = {}

            def emit_finish_a(g):
                # Z^T via K=1 matmuls (the matmul transposes a row into a
                # column), one [128,8] recip, then the outproj matmuls
                q0 = g * QG
                zt = stp.tile([128, 8], F32, tag="st", bufs=3,
                              name=f"zt_{g}")
                for j in range(8):
                    nc.tensor.matmul(zt[:, j:j + 1],
                                     zrow[g][0:1, j * 128:(j + 1) * 128],
                                     onesb[0:1, 0:1], start=True, stop=True)
                rz8 = rzp.tile([128, 8], F32, tag="rz8", name=f"rz8_{g}")
                nc.vector.reciprocal(rz8[:], zt[:])
                gp = stp.tile([128, 2 * QG], F32, tag="st", bufs=3,
                              name=f"gps_{g}")
                for i in range(4):
                    for h in range(2):
                        nc.tensor.matmul(
                            gp[:, h * QG + i * 128:h * QG + (i + 1) * 128],
                            rt[h * HC:(h + 1) * HC,
                               q0 + i * 128:q0 + (i + 1) * 128],
                            wob[h * HC:(h + 1) * HC, :],
                            start=True, stop=True)
                ob = osbp.tile([128, 512], F32, tag="osb", name=f"ob_{g}")
                last = g == NQG - 1
                for i in range(4):   # independent first halves (h0 + bias)
                    nc.vector.scalar_tensor_tensor(
                        ob[:, i * 128:(i + 1) * 128],
                        gp[:, i * 128:(i + 1) * 128],
                        rz8[:, i:i + 1], bias_bc[:],
                        mybir.AluOpType.mult, mybir.AluOpType.add)
                for i in range(4):   # second halves, chain distance 4 ops
                    obi = ob[:, i * 128:(i + 1) * 128]
                    nc.vector.scalar_tensor_tensor(
                        obi, gp[:, QG + i * 128:QG + (i + 1) * 128],
                        rz8[:, 4 + i:5 + i], obi,
                        mybir.AluOpType.mult, mybir.AluOpType.add)
                    if last:    # per-block DMA so the tail drains early,
                        # alternating issue queues to overlap descriptor gen
                        eng = nc.sync if i % 2 == 0 else nc.gpsimd
                        eng.dma_start(
                            out_d[q0 + i * 128:q0 + (i + 1) * 128, :], obi)
                if not last:
                    nc.sync.dma_start(
                        out_d[q0:q0 + QG, :].rearrange(
                            "(c r) w -> r c w", r=128),
                        ob[:].rearrange("p (c w) -> p c w", w=128))

            def emit_st_exp(g, kc, t):
                st = stp.tile([128, 2 * QG], F32, tag="st", bufs=3,
                              name=f"st_{g}_{kc}")
                if g == 0 and kc == 0:
                    ktt, ks = ktf, slice(0, 128)
                else:
                    ktt = kt[kc // 4]
                    ks = slice((kc % 4) * 128, (kc % 4 + 1) * 128)
                for h in range(2):
                    hp = slice(h * HC, (h + 1) * HC)
                    nc.tensor.matmul(
                        st[:, h * QG:(h + 1) * QG],
                        ktt[hp, ks], qt[g][hp, :],
                        start=True, stop=True)
                et = etp.tile([128, 2 * QG], BF16, tag="et",
                              name=f"et_{g}_{kc}")
                if kc in (DVE_KC_LAST if g == NQG - 1 else DVE_KC):
                    nc.vector.tensor_scalar(
                        et[:].bitcast(I16), st[:],
                        float(A_MAGIC), float(B_MAGIC),
                        mybir.AluOpType.mult, mybir.AluOpType.add)
                else:
                    nc.scalar.activation(et[:], st[:], Exp, scale=SCALE)
                ets[t] = et

            def emit_pv(p):
                g, pk = p // NKC, p % NKC
                if p % NKC == 0:
                    o_ps[g] = opp.tile([128, 2 * QG], F32, tag="ops",
                                       name=f"o_ps_{g}")
                et = ets.pop(p)
                for h in range(2):
                    nc.tensor.matmul(
                        o_ps[g][:, h * QG:(h + 1) * QG], v4[pk][:, h, :],
                        et[:, h * QG:(h + 1) * QG],
                        start=(pk == 0), stop=(pk == NKC - 1))

            # PV(p) emission step: lag LAG behind ST, but never before
            # within-group step FIRST_PV (o_ps single-buffer WAR gap)
            pv_at = {}
            for p in range(NQG * NKC):
                t = max(p + LAG, (p // NKC) * NKC + FIRST_PV)
                pv_at.setdefault(t, []).append(p)

            # STs are emitted in batches of 3 then their PVs: ST-pair ->
            # ST-pair transitions chain at the PE issue floor (~215ns) while
            # mixed PV<->ST transitions each cost ~+100ns of array turnaround
            T_END = NQG * NKC + LAG
            for t0 in range(0, T_END, 3):
                batch = range(t0, min(t0 + 3, T_END))
                for t in batch:
                    if t < NQG * NKC:
                        emit_st_exp(t // NKC, t % NKC, t)
                for t in batch:
                    g, kc = t // NKC, t % NKC
                    for p in pv_at.get(t, ()):
                        emit_pv(p)
                        if p % NKC == NKC - 1:  # group fully summed
                            emit_drain(p // NKC)
                    if t == 30:
                        nc.vector.tensor_copy(wob[:], wo32[:])
                    if t == 34:
                        bps = stp.tile([128, C], F32, tag="st", bufs=3,
                                       name="bps")
                        nc.tensor.matmul(bps[:], ones1[:], bo32[:],
                                         start=True, stop=True)
                        nc.vector.tensor_copy(bias_bc[:], bps[:])
                    if t < NQG * NKC and g == 0 and t < 28:
                        emit_proj_piece(1 + t // 4, t % 4)
                    if t < NQG * NKC and g >= 1 and kc in (8, 9):
                        emit_drain_rt(g - 1, kc - 8)
                    if t < NQG * NKC and g >= 1 and kc == 12:
                        emit_finish_a(g - 1)
            emit_finish_a(NQG - 1)

    nc.compile()
    return nc


def _prep_in_maps(x, w_qkv, w_out, b_out):
    x = np.asarray(x, dtype=np.float32).reshape(B, NPIX, C)
    w_qkv = np.asarray(w_qkv, dtype=np.float32)
    w_out = np.asarray(w_out, dtype=np.float32)
    b_out = np.asarray(b_out, dtype=np.float32)

    wq = np.ascontiguousarray(
        np.concatenate([w_qkv[:, 0:64], w_qkv[:, 192:256]], axis=1))
    wk = np.ascontiguousarray(
        np.concatenate([w_qkv[:, 64:128], w_qkv[:, 256:320]], axis=1))
    wv = np.ascontiguousarray(
        np.concatenate([w_qkv[:, 128:192], w_qkv[:, 320:384]], axis=1))
    wo = np.ascontiguousarray(w_out)
    bo = np.ascontiguousarray(b_out.reshape(1, C))

    in_maps = []
    for core in range(N_CORES):
        b, qh = core // 2, core % 2
        xbT = x[b].T                     # [C, NPIX]
        q0 = qh * NQ
        xt = np.ascontiguousarray(
            np.concatenate([xbT[:, q0:], xbT[:, :q0]], axis=1))
        in_maps.append({"xt": xt, "wq": wq, "wk": wk, "wv": wv,
                        "wo": wo, "bo": bo})
    return in_maps


def run(x, w_qkv, w_out, b_out, trace=False, **run_kwargs):
    if "nc" not in _CACHE:
        _CACHE["nc"] = _build_nc()
    nc = _CACHE["nc"]
    in_maps = _prep_in_maps(x, w_qkv, w_out, b_out)
    res = run_bass_kernel_spmd(nc, in_maps, core_ids=list(range(N_CORES)),
                               trace=trace, **run_kwargs)
    out = np.empty((B, NPIX, C), dtype=np.float32)
    for core in range(N_CORES):
        b, qh = core // 2, core % 2
        out[b, qh * NQ:(qh + 1) * NQ, :] = res.results[core]["out"]
    return out.reshape(B, 64, 64, C), res


def kernel(x, w_qkv, w_out, b_out):
    out, _ = run(x, w_qkv, w_out, b_out, trace=False)
    return out
